# revision 33
# baseline (speedup 1.0000x reference)
"""Multi-head attention (RoPE, causal) Bass kernel for 8 TRN2 NeuronCores.

Problem: x[2,2048,1024], 16 heads x 64 dim, causal mask, RoPE, f32.

Sharding: batch x head-group. Core c handles batch c//4 and the 4 heads
[4*(c%4), 4*(c%4)+4). Each core computes q/k/v projections for its head
slice, RoPE, causal attention, and a partial output projection against its
rows of Wo.T. The host sums the 4 partials per batch (the "all-reduce" of
the row-split output projection is done on the host during unsharding).

Schedule: the whole kernel is one interleaved stream. The softmax Exp on
the Scalar engine is nearly as loaded as the PE (~80us of exp vs ~100us
of matmul), so attention starts as early as possible (query block j=0
right after the first 512 cols of q/k/v are projected) and all remaining
projection / output-projection work is chopped into small "filler" units
pumped into the PE queue between attention i-blocks. The PE stays dense
(no HAM cold-downs) while ACT exps run concurrently; per-block PV waits
on exp are absorbed by fillers.

DMA: each trigger queue only sustains ~4 outstanding DMAs (completion-
semaphore recycling) ~= 90GB/s per queue, so inputs are spread in
need-order across all three queues (scalar HWDGE: wq + 2 xT-upper rows;
sync HWDGE: xT lower half + mid-kernel out writes; gpsimd SWDGE:
weights/tables + 2 xT-upper rows). gpsimd's trigger queue must retire
before the first softmax-norm partition_broadcast or the broadcast is
head-of-line blocked (measured 17us). Out writes alternate gpsimd/sync;
the tail splits triggers across sync+gpsimd and drains across DVE+ACT.
Measured dead ends, do not revisit: rerouting tail triggers off sync;
64KB mid-kernel out chunks (doubles gpsimd trigger count); pair-split
last-quarter out-proj with host-summed partial (tail savings eaten by
extra DVE/PE load in the attn(3,1) window); any tensor op on GpSimd
(trim mul +105us, RoPE tail +38us); PE identity-matmul causal mask
(per-mask LDWEIGHTS); fp8 (max-err metric would exceed the 2e-2 gate).

Device layout notes:
- x pre-transposed per batch: xT [1024, 2048] bf16 streams as the moving
  operand (bf16 halves DMA vs f32 at full matmul rate).
- Wq/Wk rows permuted per head to [e0..e15, o0..o15, e16..e31, o16..o31]
  so RoPE rotate-half is one 16<->16 DVE stream_shuffle per 32-group.
- bf16 everywhere with f32 PSUM accumulation; output written bf16 (host
  sums partials in f32).
- Score matmuls for the two heads of a pair run CONCURRENTLY in the PE
  array (K=64 row-tiles at base partitions 0/64 -> row_grp h0/h64).
- Causal masking: diagonal 128x128 blocks exp'd unmasked then multiplied
  by a 0/1 lower-triangular bf16 mask on the DVE.
- Softmax denominators ride the PV matmul: v groups are [1 | 63 zeros |
  v_h] so PV psum row 0 is the denominator, read directly from PSUM by
  reciprocal_approx_fast (its lowering requires partition 0), and rows
  64:128 are the output (DVE >32-partition windows must start at 0/64).
  Normalization: fast-reciprocal (DVE) + gpsimd partition_broadcast.
- GpSimd runs ONLY SWDGE triggers, broadcasts, and memsets.
- PSUM: scores 2x[128,2,512] (4 banks) + pv accum [128,512]x2 (2 banks)
  + shared proj/outproj [128,512]x2 (2 banks) = 8 banks exactly; the
  out-proj tail also recycles the idle scores banks.
"""

import numpy as np
import ml_dtypes

import concourse.bass as bass
import concourse.mybir as mybir
import concourse.tile as tile
from concourse import bacc
from concourse.bass_utils import run_bass_kernel_spmd

F32 = mybir.dt.float32
BF16 = mybir.dt.bfloat16

B, S, D = 2, 2048, 1024
H, HD = 16, 64
NCORES = 8
HPC = 4          # heads per core
DQ = HPC * HD    # 256 projected dims per core
THETA = 10000.0

_cached = {}


def build_nc():
    """Build the single-core Bass graph (same NEFF runs SPMD on all 8)."""
    nc = bacc.Bacc("TRN2", target_bir_lowering=False)

    xt_d = nc.dram_tensor("xt", [D, S], BF16, kind="ExternalInput")
    wq_d = nc.dram_tensor("wq", [D, DQ], BF16, kind="ExternalInput")
    wk_d = nc.dram_tensor("wk", [D, DQ], BF16, kind="ExternalInput")
    wv_d = nc.dram_tensor("wv", [D, DQ], BF16, kind="ExternalInput")
    wo_d = nc.dram_tensor("wo", [DQ, D], BF16, kind="ExternalInput")
    cos_d = nc.dram_tensor("cos", [128, S], BF16, kind="ExternalInput")
    sin_d = nc.dram_tensor("sin", [128, S], BF16, kind="ExternalInput")
    trim_d = nc.dram_tensor("trim", [128, 256], BF16, kind="ExternalInput")
    out_d = nc.dram_tensor("out", [S, D], BF16, kind="ExternalOutput")

    Exp = mybir.ActivationFunctionType.Exp
    SHUF = [(i + 16) % 32 for i in range(32)]  # 16<->16 swap per 32-group

    with tile.TileContext(nc) as tc:
        with (
            tc.tile_pool(name="consts", bufs=1) as consts,
            tc.tile_pool(name="big", bufs=8) as bigp,
            tc.tile_pool(name="qk", bufs=1) as qkp,
            tc.tile_pool(name="vsb", bufs=1) as vp,
            tc.tile_pool(name="rope", bufs=4) as ropep,
            tc.tile_pool(name="probs", bufs=6) as probsp,
            tc.tile_pool(name="small", bufs=3) as smallp,
            tc.tile_pool(name="pos", bufs=4) as posp,
            tc.tile_pool(name="sc", bufs=2, space="PSUM") as scp,
            tc.tile_pool(name="pv", bufs=1, space="PSUM") as pvp,
            tc.tile_pool(name="shr", bufs=2, space="PSUM") as shrp,
        ):
            # ---- constant tiles ----
            wq_sb = consts.tile([128, 8, DQ], BF16, tag="wq")
            wk_sb = consts.tile([128, 8, DQ], BF16, tag="wk")
            wv_sb = consts.tile([128, 8, DQ], BF16, tag="wv")
            wo_sb = consts.tile([128, 2, D], BF16, tag="wo")
            cs = {
                "cos": consts.tile([128, S], BF16, tag="cos", name="cos"),
                "sin": consts.tile([128, S], BF16, tag="sin", name="sin"),
            }
            trim_sb = consts.tile([128, 2, 128], BF16, tag="trim")

            # ---- xT resident tiles ----
            xt = []
            for k in range(8):
                t = bigp.tile([128, S], BF16, tag="big", name=f"xt{k}")
                xt.append(t)

            # ---- DMA schedule ----
            # Each trigger queue only sustains ~4 outstanding DMAs (semaphore
            # recycling), i.e. ~90GB/s per queue. Spread the input set across
            # all three queues in need-order, and keep the scalar queue free
            # of triggers once the exp era begins.
            # scalar HWDGE: wq only (gates the very first projection).
            for k in range(8):
                nc.scalar.dma_start(out=wq_sb[:, k, :],
                                    in_=wq_d[128 * k:128 * (k + 1), :])
            for k in (2, 3):
                nc.scalar.dma_start(out=xt[k][:, 1024:2048],
                                    in_=xt_d[128 * k:128 * (k + 1), 1024:2048])
            # sync HWDGE: xT lower half; idle mid-kernel; tail out writes.
            for k in range(8):
                nc.sync.dma_start(out=xt[k][:, 0:512],
                                  in_=xt_d[128 * k:128 * (k + 1), 0:512])
            for k in range(8):
                nc.sync.dma_start(out=xt[k][:, 512:1024],
                                  in_=xt_d[128 * k:128 * (k + 1), 512:1024])
            for k in (4, 5, 6, 7):
                nc.sync.dma_start(out=xt[k][:, 1024:2048],
                                  in_=xt_d[128 * k:128 * (k + 1), 1024:2048])
            # gpsimd SWDGE (8 outstanding): cos/sin, wk, wv, trim, xt upper
            # half, wo, remaining cos/sin; later most out writes.
            for name in ("cos", "sin"):
                d = cos_d if name == "cos" else sin_d
                nc.gpsimd.dma_start(out=cs[name][:, 0:512], in_=d[:, 0:512])
            for k in range(8):
                nc.gpsimd.dma_start(out=wk_sb[:, k, :],
                                    in_=wk_d[128 * k:128 * (k + 1), :])
            for k in range(4):
                nc.gpsimd.dma_start(
                    out=wv_sb[:, 2 * k:2 * k + 2, :],
                    in_=wv_d[256 * k:256 * (k + 1), :].rearrange(
                        "(k p) m -> p k m", p=128))
            nc.gpsimd.dma_start(out=trim_sb,
                                in_=trim_d.rearrange("p (h c) -> p h c", c=128))
            for name in ("cos", "sin"):
                d = cos_d if name == "cos" else sin_d
                nc.gpsimd.dma_start(out=cs[name][:, 512:1024],
                                    in_=d[:, 512:1024])
            for k in (0, 1):
                nc.gpsimd.dma_start(out=xt[k][:, 1024:2048],
                                    in_=xt_d[128 * k:128 * (k + 1), 1024:2048])
            for name in ("cos", "sin"):
                d = cos_d if name == "cos" else sin_d
                nc.gpsimd.dma_start(out=cs[name][:, 1024:1536],
                                    in_=d[:, 1024:1536])
                nc.gpsimd.dma_start(out=cs[name][:, 1536:2048],
                                    in_=d[:, 1536:2048])
            nc.gpsimd.dma_start(out=wo_sb,
                                in_=wo_d.rearrange("(k p) m -> p k m", p=128))

            # warmup exp: pulls the ACT table load (~2.7us) forward so it
            # doesn't land in front of the first real softmax. Reads garbage
            # (no writer) on purpose - output is unused.
            warm = smallp.tile([1, 64], F32, tag="warm")
            nc.scalar.activation(warm, warm, Exp)

            # q/k destination tiles: [pair][128 rows = 2 heads x 64, S]
            qt = [qkp.tile([128, S], BF16, tag=f"qt{p}", name=f"qt{p}") for p in range(2)]
            kt = [qkp.tile([128, S], BF16, tag=f"kt{p}", name=f"kt{p}") for p in range(2)]
            # v tiles: per s-chunk [128, 4*128] ([1 | 63 zeros | v_h] per
            # head: PV psum row 0 = denominator, rows 64:128 = output; DVE
            # partition windows >32 wide must start at 0 or 64)
            vsb = [vp.tile([128, 4 * 128], BF16, tag=f"v{i}", name=f"v{i}") for i in range(16)]
            # attention output (pre out-proj): [pair][128 = 2 heads x 64 dv, S]
            ot = [qkp.tile([128, S], BF16, tag=f"ot{p}", name=f"ot{p}") for p in range(2)]

            # ---- building blocks ----
            def proj_qk_half(w_sb, dst, cosn, sinn, m, n, half, dve_only=False):
                """Project + RoPE one 512-col window for one pair.

                RoPE reads the proj psum directly (no staging cast); the
                sin-mul and final add run on the otherwise idle GpSimd so
                the DVE queue stays short (it gates norm chains and next-
                pair scores). Preroll uses dve_only=True: gpsimd still has
                parked SWDGE triggers then.
                """
                ps = shrp.tile([128, 512], F32, tag="shr", name="ps")
                mcol = slice(128 * m, 128 * (m + 1))
                wcol = slice(1024 * n + 512 * half, 1024 * n + 512 * (half + 1))
                for k in range(8):
                    nc.tensor.matmul(
                        ps,
                        lhsT=w_sb[:, k, mcol],
                        rhs=xt[k][:, wcol],
                        start=(k == 0),
                        stop=(k == 7),
                    )
                eng2 = nc.vector
                raw = ropep.tile([128, 512], BF16, tag="raw")
                nc.vector.tensor_copy(raw, ps)
                rot = ropep.tile([128, 512], BF16, tag="rot")
                nc.vector.stream_shuffle(rot, raw, SHUF)
                t1 = ropep.tile([128, 512], BF16, tag="t1")
                nc.vector.tensor_mul(t1, raw, cs[cosn][:, wcol])
                t2 = ropep.tile([128, 512], BF16, tag="t2")
                eng2.tensor_mul(t2, rot, cs[sinn][:, wcol])
                eng2.tensor_add(dst[m][:, wcol], t1, t2)

            def proj_v_2sub(n, sub2):
                """v for 2 s-chunks (256 tokens) into one psum bank."""
                psv = shrp.tile([128, 512], F32, tag="shr", name="psv")
                for sub in range(2):
                    i = 8 * n + 2 * sub2 + sub
                    scol = slice(128 * i, 128 * (i + 1))
                    half = slice(256 * sub, 256 * sub + 256)
                    for k in range(8):
                        nc.tensor.matmul(
                            psv[:, half],
                            lhsT=xt[k][:, scol],
                            rhs=wv_sb[:, k, :],
                            start=(sub == 0 and k == 0),
                            stop=(sub == 1 and k == 7),
                        )
                for sub in range(2):
                    i = 8 * n + 2 * sub2 + sub
                    half = slice(256 * sub, 256 * sub + 256)
                    vr = vsb[i].rearrange("p (h c) -> p h c", c=128)
                    nc.vector.memset(vr[:, :, 0], 1.0)
                    nc.vector.memset(vr[:, :, 1:64], 0.0)
                    # drain on ACT: plenty of exp slack in the early windows,
                    # and it keeps the congested DVE queue short
                    nc.scalar.copy(
                        vr[:, :, 64:128],
                        psv[:, half].rearrange("p (h c) -> p h c", c=64),
                    )

            # ---- filler machinery: deferred emission closures ----
            fillers = []
            pump_acc = [0.0]

            def pump_frac(frac):
                pump_acc[0] += frac
                while fillers and pump_acc[0] >= 1.0:
                    pump_acc[0] -= 1.0
                    fillers.pop(0)()

            def pump_all():
                while fillers:
                    fillers.pop(0)()
                pump_acc[0] = 0.0

            # ---- output projection units ----
            def emit_po_md(m, d, last):
                """Out-proj for query block m, 512-col half d."""
                if last and (2 * m + d) % 2:
                    # tail: the scores pool is idle after the last exp; use
                    # its banks so 4 out-proj tiles pipeline instead of 2
                    po = scp.tile([128, 2, 512], F32, tag="sc", name="po")[:, 0, :]
                else:
                    po = shrp.tile([128, 512], F32, tag="shr", name="po")
                for pp in range(2):
                    nc.tensor.matmul(
                        po,
                        lhsT=ot[pp][:, 128 * m:128 * (m + 1)],
                        rhs=wo_sb[:, pp, 512 * d:512 * (d + 1)],
                        start=(pp == 0),
                        stop=(pp == 1),
                    )
                posb = posp.tile([128, 512], BF16, tag="pos", name="posb")
                # tail units alternate ACT/DVE drains (both idle by then);
                # mid-kernel units stay on DVE so they don't delay exps
                if last and (2 * m + d) % 2:
                    nc.scalar.copy(posb, po)
                else:
                    nc.vector.tensor_copy(posb, po)
                rows = slice(128 * m, 128 * (m + 1))
                if last:
                    # final blocks: 64KB chunks, triggers split across the
                    # sync and gpsimd queues for a short tail drain
                    for q in range(2):
                        cl = 256 * q
                        eng = nc.sync if q % 2 == 0 else nc.gpsimd
                        eng.dma_start(
                            out=out_d[rows, 512 * d + cl:512 * d + cl + 256],
                            in_=posb[:, cl:cl + 256])
                else:
                    # mid-kernel out writes: single trigger, alternating
                    # gpsimd/sync so neither queue backs up before the tail
                    eng = nc.gpsimd if (2 * m + d) % 2 else nc.sync
                    eng.dma_start(out=out_d[rows, 512 * d:512 * (d + 1)],
                                  in_=posb)

            # ---- attention ----
            def mk_norm(p, j, pva, pvb):
                """Normalize both heads of pair p for query block j.

                PV psum row 0 is the denominator ([1|v] layout), so the
                fast-reciprocal reads it straight from PSUM (partition 0 as
                its lowering requires) with no staging copy.
                """
                jcol = slice(512 * j, 512 * (j + 1))
                for h, pvt in ((0, pva), (1, pvb)):
                    rr = smallp.tile([1, 512], F32, tag=f"rr{h}", name="rr")
                    nc.vector.reciprocal_approx_fast(rr, pvt[0:1, :])
                    rdb = smallp.tile([64, 512], F32, tag=f"rdb{h}", name="rdb")
                    nc.gpsimd.partition_broadcast(rdb, rr)
                    nc.vector.tensor_mul(
                        ot[p][64 * h:64 * (h + 1), jcol],
                        pvt[64:128, :],
                        rdb,
                    )

            def attn_pair(j, p, fill_per_block):
                pva = pvp.tile([128, 512], F32, tag="pva", name="pva")
                pvb = pvp.tile([128, 512], F32, tag="pvb", name="pvb")
                pv = (pva, pvb)
                nlast = 4 * j + 3

                def emit_pv(pend):
                    i, probs, loc = pend
                    for h in range(2):
                        hh = 2 * p + h
                        nc.tensor.matmul(
                            pv[h][:, loc:512],
                            lhsT=vsb[i][:, 128 * hh:128 * hh + 128],
                            rhs=probs[:, h, loc:512],
                            start=(i == 0),
                            stop=(i == nlast),
                        )

                # one-deep software pipeline: the PE issues scores(i+1)
                # before PV(i), so the softmax Exp latency of block i hides
                # under score matmuls + filler work of block i+1.
                pend = None
                for i in range(4 * j + 4):
                    r = i - 4 * j
                    loc = max(0, 128 * r)
                    sc = scp.tile([128, 2, 512], F32, tag="sc")
                    icol = slice(128 * i, 128 * (i + 1))
                    for h in range(2):
                        rows = slice(64 * h, 64 * (h + 1))
                        nc.tensor.matmul(
                            sc[:, h, loc:512],
                            lhsT=kt[p][rows, icol],
                            rhs=qt[p][rows, 512 * j + loc:512 * (j + 1)],
                            start=True,
                            stop=True,
                        )
                    probs = probsp.tile([128, 2, 512], BF16, tag="probs")
                    nc.scalar.activation(
                        probs[:, :, loc:512], sc[:, :, loc:512], Exp
                    )
                    if r >= 0:
                        nc.vector.tensor_mul(
                            probs[:, :, loc:loc + 128],
                            probs[:, :, loc:loc + 128],
                            trim_sb,
                        )
                    pump_frac(fill_per_block)
                    if pend is not None:
                        emit_pv(pend)
                    pend = (i, probs, loc)
                emit_pv(pend)
                mk_norm(p, j, pva, pvb)

            # ---- pre-roll: q/k for pair 0, tokens 0:512 (minimal) ----
            proj_qk_half(wq_sb, qt, "cos", "sin", 0, 0, 0, dve_only=True)
            proj_qk_half(wk_sb, kt, "cos", "sin", 0, 0, 0, dve_only=True)

            # ---- filler schedule (deferred emission, need-ordered) ----
            # attn(0,1) needs q/k(m=1,h0) + vsb[0:4]; attn(1,*) needs h1 of
            # n=0 and vsb[4:8]; attn(2,*) needs q/k n=1 (keys to 1536) and
            # vsb[8:12]; attn(3,*) needs the rest of k n=1 and vsb[12:16].
            # out-proj units for query block group j unlock after attn(j,1).
            def F(fn, *a):
                return lambda: fn(*a)

            sched = {
                (0, 0): [F(proj_v_2sub, 0, 0),
                         F(proj_qk_half, wq_sb, qt, "cos", "sin", 1, 0, 0),
                         F(proj_qk_half, wk_sb, kt, "cos", "sin", 1, 0, 0),
                         F(proj_v_2sub, 0, 1)],
                (0, 1): [F(proj_qk_half, wq_sb, qt, "cos", "sin", 0, 0, 1),
                         F(proj_qk_half, wk_sb, kt, "cos", "sin", 0, 0, 1),
                         F(proj_qk_half, wq_sb, qt, "cos", "sin", 1, 0, 1),
                         F(proj_qk_half, wk_sb, kt, "cos", "sin", 1, 0, 1)],
                (1, 0): [F(proj_v_2sub, 0, 2), F(proj_v_2sub, 0, 3),
                         F(proj_qk_half, wq_sb, qt, "cos", "sin", 0, 1, 0),
                         F(proj_qk_half, wq_sb, qt, "cos", "sin", 1, 1, 0)],
                (1, 1): [F(proj_qk_half, wk_sb, kt, "cos", "sin", 0, 1, 0),
                         F(proj_qk_half, wk_sb, kt, "cos", "sin", 1, 1, 0),
                         F(proj_qk_half, wq_sb, qt, "cos", "sin", 0, 1, 1),
                         F(proj_qk_half, wq_sb, qt, "cos", "sin", 1, 1, 1)],
                (2, 0): [F(proj_v_2sub, 1, 0), F(proj_v_2sub, 1, 1),
                         F(proj_qk_half, wk_sb, kt, "cos", "sin", 0, 1, 1),
                         F(proj_qk_half, wk_sb, kt, "cos", "sin", 1, 1, 1),
                         F(emit_po_md, 0, 0, False), F(emit_po_md, 0, 1, False)],
                (2, 1): [F(proj_v_2sub, 1, 2), F(proj_v_2sub, 1, 3),
                         F(emit_po_md, 1, 0, False), F(emit_po_md, 1, 1, False),
                         F(emit_po_md, 2, 0, False), F(emit_po_md, 2, 1, False)],
                (3, 0): [F(emit_po_md, 3, 0, False), F(emit_po_md, 3, 1, False)]
                        + [F(emit_po_md, m, d, False)
                           for m in range(4, 7) for d in range(2)],
                (3, 1): [F(emit_po_md, m, d, False)
                         for m in range(7, 12) for d in range(2)],
            }

            for j in range(4):
                for p in range(2):
                    fillers.extend(sched[(j, p)])
                    nblocks = 4 * j + 4
                    attn_pair(j, p, len(fillers) / nblocks)
                    pump_all()

            # final out-proj for query blocks 12..15
            for m in range(12, 16):
                for dd in range(2):
                    emit_po_md(m, dd, True)

    nc.compile()
    return nc


def _host_inputs(x, Wq, Wk, Wv, Wo, token_positions):
    """Build per-core input maps (all host-side numpy prep)."""
    bf = ml_dtypes.bfloat16
    x = np.asarray(x, dtype=np.float32)
    Wq = np.asarray(Wq, dtype=np.float32)
    Wk = np.asarray(Wk, dtype=np.float32)
    Wv = np.asarray(Wv, dtype=np.float32)
    Wo = np.asarray(Wo, dtype=np.float32)
    pos = np.asarray(token_positions).astype(np.float64)

    # RoPE tables in the permuted-lane layout (16-lane e/o blocks).
    idx = np.arange(0, HD, 2, dtype=np.float64) / HD
    freqs = 1.0 / THETA ** idx                      # [32]
    ang = pos[:, None] * freqs[None, :]             # [S, 32]
    c, s = np.cos(ang).T, np.sin(ang).T             # [32, S]
    c64 = np.concatenate([c[0:16], c[0:16], c[16:32], c[16:32]], 0)
    s64 = np.concatenate([-s[0:16], s[0:16], -s[16:32], s[16:32]], 0)
    cosb = np.concatenate([c64, c64], 0).astype(bf)
    sinb = np.concatenate([s64, s64], 0).astype(bf)

    # 0/1 keep-mask for the diagonal block: keep keys (rows) <= query (cols),
    # duplicated for both heads of a pair.
    tri01 = (np.arange(128)[:, None] <= np.arange(128)[None, :]).astype(bf)
    trim = np.concatenate([tri01, tri01], axis=1)   # [128, 256]

    # per-head row permutation: [e0..e15, o0..o15, e16..e31, o16..o31]
    perm64 = np.concatenate([
        np.arange(0, 32, 2), np.arange(1, 32, 2),
        np.arange(32, 64, 2), np.arange(33, 64, 2),
    ])

    xts = [np.ascontiguousarray(x[b].T).astype(bf) for b in range(B)]

    in_maps = []
    for core in range(NCORES):
        b = core // 4
        heads = [4 * (core % 4) + hh for hh in range(HPC)]
        qk_rows = np.concatenate([g * HD + perm64 for g in heads])
        v_rows = np.concatenate([np.arange(g * HD, (g + 1) * HD) for g in heads])
        in_maps.append({
            "xt": xts[b],
            "wq": (np.ascontiguousarray(Wq[qk_rows, :].T) / np.sqrt(HD)).astype(bf),
            "wk": np.ascontiguousarray(Wk[qk_rows, :].T).astype(bf),
            "wv": np.ascontiguousarray(Wv[v_rows, :].T).astype(bf),
            "wo": np.ascontiguousarray(Wo[:, v_rows].T).astype(bf),
            "cos": cosb, "sin": sinb,
            "trim": trim,
        })
    return in_maps


def _ensure_ntff_hook():
    """Register the axon NTFF profile hook if the image's antenv lacks it."""
    import sys, types
    try:
        import antenv.axon_hooks  # noqa: F401
        return
    except ImportError:
        pass
    try:
        from trn_agent_boot.trn_boot import _ntff_profile_via_ctypes
        hook = _ntff_profile_via_ctypes("/opt/axon/libaxon_pjrt.so")
    except Exception:
        return
    mod = types.ModuleType("antenv.axon_hooks")
    mod.get_axon_ntff_profile_hook = lambda: hook
    mod.set_axon_ntff_profile_hook = lambda h: None
    sys.modules["antenv.axon_hooks"] = mod


def run(inputs, trace=False):
    """Run the SPMD kernel; returns (full_output, BassKernelResults)."""
    if trace:
        _ensure_ntff_hook()
    if "nc" not in _cached:
        _cached["nc"] = build_nc()
    nc = _cached["nc"]
    in_maps = _host_inputs(
        inputs["x"], inputs["Wq"], inputs["Wk"], inputs["Wv"], inputs["Wo"],
        inputs["token_positions"],
    )
    res = run_bass_kernel_spmd(nc, in_maps, core_ids=list(range(NCORES)),
                               trace=trace)
    out = np.zeros((B, S, D), dtype=np.float32)
    for core in range(NCORES):
        out[core // 4] += res.results[core]["out"].astype(np.float32)
    return out, res


def kernel(**inputs) -> np.ndarray:
    out, _ = run(inputs, trace=False)
    return out


# revision 36
# speedup vs baseline: 1.0196x; 1.0196x over previous
"""Multi-head attention (RoPE, causal) Bass kernel for 8 TRN2 NeuronCores.

Problem: x[2,2048,1024], 16 heads x 64 dim, causal mask, RoPE, f32.

Sharding: batch x head-group. Core c handles batch c//4 and the 4 heads
[4*(c%4), 4*(c%4)+4). Each core computes q/k/v projections for its head
slice, RoPE, causal attention, and a partial output projection against its
rows of Wo.T. The host sums the 4 partials per batch (the "all-reduce" of
the row-split output projection is done on the host during unsharding).

Schedule: the whole kernel is one interleaved stream. The softmax Exp on
the Scalar engine is nearly as loaded as the PE (~80us of exp vs ~100us
of matmul), so attention starts as early as possible (query block j=0
right after the first 512 cols of q/k/v are projected) and all remaining
projection / output-projection work is chopped into small "filler" units
pumped into the PE queue between attention i-blocks. The PE stays dense
(no HAM cold-downs) while ACT exps run concurrently; per-block PV waits
on exp are absorbed by fillers.

DMA: each trigger queue only sustains ~4 outstanding DMAs (completion-
semaphore recycling) ~= 90GB/s per queue, so inputs are spread in
need-order across all three queues (scalar HWDGE: wq + 2 xT-upper rows;
sync HWDGE: xT lower half + mid-kernel out writes; gpsimd SWDGE:
weights/tables + 2 xT-upper rows). gpsimd's trigger queue must retire
before the first softmax-norm partition_broadcast or the broadcast is
head-of-line blocked (measured 17us). Out writes alternate gpsimd/sync;
the tail splits triggers across sync+gpsimd and drains across DVE+ACT.
Measured dead ends, do not revisit: rerouting tail triggers off sync;
64KB mid-kernel out chunks (doubles gpsimd trigger count); pair-split
last-quarter out-proj with host-summed partial (tail savings eaten by
extra DVE/PE load in the attn(3,1) window); any tensor op on GpSimd
(trim mul +105us, RoPE tail +38us); PE identity-matmul causal mask
(per-mask LDWEIGHTS); fp8 (max-err metric would exceed the 2e-2 gate).

Device layout notes:
- x pre-transposed per batch: xT [1024, 2048] bf16 streams as the moving
  operand (bf16 halves DMA vs f32 at full matmul rate).
- Wq/Wk rows permuted per head to [e0..e15, o0..o15, e16..e31, o16..o31]
  so RoPE rotate-half is one 16<->16 DVE stream_shuffle per 32-group.
- bf16 everywhere with f32 PSUM accumulation; output written bf16 (host
  sums partials in f32).
- Score matmuls for the two heads of a pair run CONCURRENTLY in the PE
  array (K=64 row-tiles at base partitions 0/64 -> row_grp h0/h64).
- Causal masking: diagonal 128x128 blocks exp'd unmasked then multiplied
  by a 0/1 lower-triangular bf16 mask on the DVE.
- Softmax denominators ride the PV matmul: v groups are [1 | 63 zeros |
  v_h] so PV psum row 0 is the denominator, read directly from PSUM by
  reciprocal_approx_fast (its lowering requires partition 0), and rows
  64:128 are the output (DVE >32-partition windows must start at 0/64).
  Normalization: fast-reciprocal (DVE) + gpsimd partition_broadcast.
- GpSimd runs ONLY SWDGE triggers, broadcasts, and memsets.
- PSUM: scores 2x[128,2,512] (4 banks) + pv accum [128,512]x2 (2 banks)
  + shared proj/outproj [128,512]x2 (2 banks) = 8 banks exactly; the
  out-proj tail also recycles the idle scores banks.
"""

import numpy as np
import ml_dtypes

import concourse.bass as bass
import concourse.mybir as mybir
import concourse.tile as tile
from concourse import bacc
from concourse.bass_utils import run_bass_kernel_spmd

F32 = mybir.dt.float32
BF16 = mybir.dt.bfloat16

B, S, D = 2, 2048, 1024
H, HD = 16, 64
NCORES = 8
HPC = 4          # heads per core
DQ = HPC * HD    # 256 projected dims per core
THETA = 10000.0

_cached = {}


def build_nc():
    """Build the single-core Bass graph (same NEFF runs SPMD on all 8)."""
    nc = bacc.Bacc("TRN2", target_bir_lowering=False)

    xt_d = nc.dram_tensor("xt", [D, S], BF16, kind="ExternalInput")
    wq_d = nc.dram_tensor("wq", [D, DQ], BF16, kind="ExternalInput")
    wk_d = nc.dram_tensor("wk", [D, DQ], BF16, kind="ExternalInput")
    wv_d = nc.dram_tensor("wv", [D, DQ], BF16, kind="ExternalInput")
    wo_d = nc.dram_tensor("wo", [DQ, D], BF16, kind="ExternalInput")
    cos_d = nc.dram_tensor("cos", [128, S], BF16, kind="ExternalInput")
    sin_d = nc.dram_tensor("sin", [128, S], BF16, kind="ExternalInput")
    trim_d = nc.dram_tensor("trim", [128, 256], BF16, kind="ExternalInput")
    out_d = nc.dram_tensor("out", [S, D], BF16, kind="ExternalOutput")

    Exp = mybir.ActivationFunctionType.Exp
    SHUF = [(i + 16) % 32 for i in range(32)]  # 16<->16 swap per 32-group

    with tile.TileContext(nc) as tc:
        with (
            tc.tile_pool(name="consts", bufs=1) as consts,
            tc.tile_pool(name="big", bufs=8) as bigp,
            tc.tile_pool(name="qk", bufs=1) as qkp,
            tc.tile_pool(name="vsb", bufs=1) as vp,
            tc.tile_pool(name="rope", bufs=4) as ropep,
            tc.tile_pool(name="probs", bufs=6) as probsp,
            tc.tile_pool(name="small", bufs=3) as smallp,
            tc.tile_pool(name="pos", bufs=4) as posp,
            tc.tile_pool(name="sc", bufs=2, space="PSUM") as scp,
            tc.tile_pool(name="pv", bufs=1, space="PSUM") as pvp,
            tc.tile_pool(name="shr", bufs=2, space="PSUM") as shrp,
        ):
            # ---- constant tiles ----
            wq_sb = consts.tile([128, 8, DQ], BF16, tag="wq")
            wk_sb = consts.tile([128, 8, DQ], BF16, tag="wk")
            wv_sb = consts.tile([128, 8, DQ], BF16, tag="wv")
            wo_sb = consts.tile([128, 2, D], BF16, tag="wo")
            cs = {
                "cos": consts.tile([128, S], BF16, tag="cos", name="cos"),
                "sin": consts.tile([128, S], BF16, tag="sin", name="sin"),
            }
            trim_sb = consts.tile([128, 2, 128], BF16, tag="trim")

            # ---- xT resident tiles ----
            xt = []
            for k in range(8):
                t = bigp.tile([128, S], BF16, tag="big", name=f"xt{k}")
                xt.append(t)

            # ---- DMA schedule ----
            # Each trigger queue only sustains ~4 outstanding DMAs (semaphore
            # recycling), i.e. ~90GB/s per queue. Spread the input set across
            # all three queues in need-order, and keep the scalar queue free
            # of triggers once the exp era begins.
            # scalar HWDGE: wq only (gates the very first projection).
            for k in range(8):
                nc.scalar.dma_start(out=wq_sb[:, k, :],
                                    in_=wq_d[128 * k:128 * (k + 1), :])
            for k in (2, 3):
                nc.scalar.dma_start(out=xt[k][:, 1024:2048],
                                    in_=xt_d[128 * k:128 * (k + 1), 1024:2048])
            # sync HWDGE: xT lower half; idle mid-kernel; tail out writes.
            for k in range(8):
                nc.sync.dma_start(out=xt[k][:, 0:512],
                                  in_=xt_d[128 * k:128 * (k + 1), 0:512])
            for k in range(8):
                nc.sync.dma_start(out=xt[k][:, 512:1024],
                                  in_=xt_d[128 * k:128 * (k + 1), 512:1024])
            for k in (4, 5, 6, 7):
                nc.sync.dma_start(out=xt[k][:, 1024:2048],
                                  in_=xt_d[128 * k:128 * (k + 1), 1024:2048])
            # gpsimd SWDGE (8 outstanding): cos/sin, wk, wv, trim, xt upper
            # half, wo, remaining cos/sin; later most out writes.
            for name in ("cos", "sin"):
                d = cos_d if name == "cos" else sin_d
                nc.gpsimd.dma_start(out=cs[name][:, 0:512], in_=d[:, 0:512])
            for k in range(8):
                nc.gpsimd.dma_start(out=wk_sb[:, k, :],
                                    in_=wk_d[128 * k:128 * (k + 1), :])
            for k in range(4):
                nc.gpsimd.dma_start(
                    out=wv_sb[:, 2 * k:2 * k + 2, :],
                    in_=wv_d[256 * k:256 * (k + 1), :].rearrange(
                        "(k p) m -> p k m", p=128))
            nc.gpsimd.dma_start(out=trim_sb,
                                in_=trim_d.rearrange("p (h c) -> p h c", c=128))
            for name in ("cos", "sin"):
                d = cos_d if name == "cos" else sin_d
                nc.gpsimd.dma_start(out=cs[name][:, 512:1024],
                                    in_=d[:, 512:1024])
            for k in (0, 1):
                nc.gpsimd.dma_start(out=xt[k][:, 1024:2048],
                                    in_=xt_d[128 * k:128 * (k + 1), 1024:2048])
            for name in ("cos", "sin"):
                d = cos_d if name == "cos" else sin_d
                nc.gpsimd.dma_start(out=cs[name][:, 1024:1536],
                                    in_=d[:, 1024:1536])
                nc.gpsimd.dma_start(out=cs[name][:, 1536:2048],
                                    in_=d[:, 1536:2048])
            nc.gpsimd.dma_start(out=wo_sb,
                                in_=wo_d.rearrange("(k p) m -> p k m", p=128))

            # warmup exp: pulls the ACT table load (~2.7us) forward so it
            # doesn't land in front of the first real softmax. Reads garbage
            # (no writer) on purpose - output is unused.
            warm = smallp.tile([1, 64], F32, tag="warm")
            nc.scalar.activation(warm, warm, Exp)

            # HAM warmup: ~20 junk matmuls run while the first DMA wave
            # lands, releasing the PE clock gate (K=4/8 cold -> 8/8 warm)
            # before the real projections start (preroll MMs measured 607ns
            # cold vs 379ns warm). Finishes ~7us in, before the first real
            # matmul's inputs arrive; nothing reads the result.
            junk = consts.tile([128, 512], BF16, tag="junk")
            nc.vector.memset(junk, 1.0)
            jps = shrp.tile([128, 512], F32, tag="shr", name="jps")
            for w in range(20):
                nc.tensor.matmul(jps, lhsT=junk[:, 0:128], rhs=junk,
                                 start=(w == 0), stop=(w == 19))

            # q/k destination tiles: [pair][128 rows = 2 heads x 64, S]
            qt = [qkp.tile([128, S], BF16, tag=f"qt{p}", name=f"qt{p}") for p in range(2)]
            kt = [qkp.tile([128, S], BF16, tag=f"kt{p}", name=f"kt{p}") for p in range(2)]
            # v tiles: per s-chunk [128, 4*128] ([1 | 63 zeros | v_h] per
            # head: PV psum row 0 = denominator, rows 64:128 = output; DVE
            # partition windows >32 wide must start at 0 or 64)
            vsb = [vp.tile([128, 4 * 128], BF16, tag=f"v{i}", name=f"v{i}") for i in range(16)]
            # attention output (pre out-proj): [pair][128 = 2 heads x 64 dv, S]
            ot = [qkp.tile([128, S], BF16, tag=f"ot{p}", name=f"ot{p}") for p in range(2)]

            # ---- building blocks ----
            def proj_qk_half(w_sb, dst, cosn, sinn, m, n, half, dve_only=False):
                """Project + RoPE one 512-col window for one pair.

                RoPE reads the proj psum directly (no staging cast); the
                sin-mul and final add run on the otherwise idle GpSimd so
                the DVE queue stays short (it gates norm chains and next-
                pair scores). Preroll uses dve_only=True: gpsimd still has
                parked SWDGE triggers then.
                """
                ps = shrp.tile([128, 512], F32, tag="shr", name="ps")
                mcol = slice(128 * m, 128 * (m + 1))
                wcol = slice(1024 * n + 512 * half, 1024 * n + 512 * (half + 1))
                for k in range(8):
                    nc.tensor.matmul(
                        ps,
                        lhsT=w_sb[:, k, mcol],
                        rhs=xt[k][:, wcol],
                        start=(k == 0),
                        stop=(k == 7),
                    )
                eng2 = nc.vector
                raw = ropep.tile([128, 512], BF16, tag="raw")
                nc.vector.tensor_copy(raw, ps)
                rot = ropep.tile([128, 512], BF16, tag="rot")
                nc.vector.stream_shuffle(rot, raw, SHUF)
                t1 = ropep.tile([128, 512], BF16, tag="t1")
                nc.vector.tensor_mul(t1, raw, cs[cosn][:, wcol])
                t2 = ropep.tile([128, 512], BF16, tag="t2")
                eng2.tensor_mul(t2, rot, cs[sinn][:, wcol])
                eng2.tensor_add(dst[m][:, wcol], t1, t2)

            def proj_v_2sub(n, sub2):
                """v for 2 s-chunks (256 tokens) into one psum bank."""
                psv = shrp.tile([128, 512], F32, tag="shr", name="psv")
                for sub in range(2):
                    i = 8 * n + 2 * sub2 + sub
                    scol = slice(128 * i, 128 * (i + 1))
                    half = slice(256 * sub, 256 * sub + 256)
                    for k in range(8):
                        nc.tensor.matmul(
                            psv[:, half],
                            lhsT=xt[k][:, scol],
                            rhs=wv_sb[:, k, :],
                            start=(sub == 0 and k == 0),
                            stop=(sub == 1 and k == 7),
                        )
                for sub in range(2):
                    i = 8 * n + 2 * sub2 + sub
                    half = slice(256 * sub, 256 * sub + 256)
                    vr = vsb[i].rearrange("p (h c) -> p h c", c=128)
                    nc.vector.memset(vr[:, :, 0], 1.0)
                    nc.vector.memset(vr[:, :, 1:64], 0.0)
                    # drain on ACT: plenty of exp slack in the early windows,
                    # and it keeps the congested DVE queue short
                    nc.scalar.copy(
                        vr[:, :, 64:128],
                        psv[:, half].rearrange("p (h c) -> p h c", c=64),
                    )

            # ---- filler machinery: deferred emission closures ----
            fillers = []
            pump_acc = [0.0]

            def pump_frac(frac):
                pump_acc[0] += frac
                while fillers and pump_acc[0] >= 1.0:
                    pump_acc[0] -= 1.0
                    fillers.pop(0)()

            def pump_all():
                while fillers:
                    fillers.pop(0)()
                pump_acc[0] = 0.0

            # ---- output projection units ----
            def emit_po_md(m, d, last):
                """Out-proj for query block m, 512-col half d."""
                if last and (2 * m + d) % 2:
                    # tail: the scores pool is idle after the last exp; use
                    # its banks so 4 out-proj tiles pipeline instead of 2
                    po = scp.tile([128, 2, 512], F32, tag="sc", name="po")[:, 0, :]
                else:
                    po = shrp.tile([128, 512], F32, tag="shr", name="po")
                for pp in range(2):
                    nc.tensor.matmul(
                        po,
                        lhsT=ot[pp][:, 128 * m:128 * (m + 1)],
                        rhs=wo_sb[:, pp, 512 * d:512 * (d + 1)],
                        start=(pp == 0),
                        stop=(pp == 1),
                    )
                posb = posp.tile([128, 512], BF16, tag="pos", name="posb")
                # tail units alternate ACT/DVE drains (both idle by then);
                # mid-kernel units stay on DVE so they don't delay exps
                if last and (2 * m + d) % 2:
                    nc.scalar.copy(posb, po)
                else:
                    nc.vector.tensor_copy(posb, po)
                rows = slice(128 * m, 128 * (m + 1))
                if last:
                    # final blocks: 64KB chunks, triggers split across the
                    # sync and gpsimd queues for a short tail drain
                    for q in range(2):
                        cl = 256 * q
                        eng = nc.sync if q % 2 == 0 else nc.gpsimd
                        eng.dma_start(
                            out=out_d[rows, 512 * d + cl:512 * d + cl + 256],
                            in_=posb[:, cl:cl + 256])
                else:
                    # mid-kernel out writes: single trigger, alternating
                    # gpsimd/sync so neither queue backs up before the tail
                    eng = nc.gpsimd if (2 * m + d) % 2 else nc.sync
                    eng.dma_start(out=out_d[rows, 512 * d:512 * (d + 1)],
                                  in_=posb)

            # ---- attention ----
            def mk_norm(p, j, pva, pvb):
                """Normalize both heads of pair p for query block j.

                PV psum row 0 is the denominator ([1|v] layout), so the
                fast-reciprocal reads it straight from PSUM (partition 0 as
                its lowering requires) with no staging copy.
                """
                jcol = slice(512 * j, 512 * (j + 1))
                for h, pvt in ((0, pva), (1, pvb)):
                    rr = smallp.tile([1, 512], F32, tag=f"rr{h}", name="rr")
                    nc.vector.reciprocal_approx_fast(rr, pvt[0:1, :])
                    rdb = smallp.tile([64, 512], F32, tag=f"rdb{h}", name="rdb")
                    nc.gpsimd.partition_broadcast(rdb, rr)
                    nc.vector.tensor_mul(
                        ot[p][64 * h:64 * (h + 1), jcol],
                        pvt[64:128, :],
                        rdb,
                    )

            def attn_pair(j, p, fill_per_block):
                pva = pvp.tile([128, 512], F32, tag="pva", name="pva")
                pvb = pvp.tile([128, 512], F32, tag="pvb", name="pvb")
                pv = (pva, pvb)
                nlast = 4 * j + 3

                def emit_pv(pend):
                    i, probs, loc = pend
                    for h in range(2):
                        hh = 2 * p + h
                        nc.tensor.matmul(
                            pv[h][:, loc:512],
                            lhsT=vsb[i][:, 128 * hh:128 * hh + 128],
                            rhs=probs[:, h, loc:512],
                            start=(i == 0),
                            stop=(i == nlast),
                        )

                # one-deep software pipeline: the PE issues scores(i+1)
                # before PV(i), so the softmax Exp latency of block i hides
                # under score matmuls + filler work of block i+1.
                pend = None
                for i in range(4 * j + 4):
                    r = i - 4 * j
                    loc = max(0, 128 * r)
                    sc = scp.tile([128, 2, 512], F32, tag="sc")
                    icol = slice(128 * i, 128 * (i + 1))
                    for h in range(2):
                        rows = slice(64 * h, 64 * (h + 1))
                        nc.tensor.matmul(
                            sc[:, h, loc:512],
                            lhsT=kt[p][rows, icol],
                            rhs=qt[p][rows, 512 * j + loc:512 * (j + 1)],
                            start=True,
                            stop=True,
                        )
                    probs = probsp.tile([128, 2, 512], BF16, tag="probs")
                    nc.scalar.activation(
                        probs[:, :, loc:512], sc[:, :, loc:512], Exp
                    )
                    if r >= 0:
                        nc.vector.tensor_mul(
                            probs[:, :, loc:loc + 128],
                            probs[:, :, loc:loc + 128],
                            trim_sb,
                        )
                    pump_frac(fill_per_block)
                    if pend is not None:
                        emit_pv(pend)
                    pend = (i, probs, loc)
                emit_pv(pend)
                mk_norm(p, j, pva, pvb)

            # ---- pre-roll: q/k for pair 0, tokens 0:512 (minimal) ----
            proj_qk_half(wq_sb, qt, "cos", "sin", 0, 0, 0, dve_only=True)
            proj_qk_half(wk_sb, kt, "cos", "sin", 0, 0, 0, dve_only=True)

            # ---- filler schedule (deferred emission, need-ordered) ----
            # attn(0,1) needs q/k(m=1,h0) + vsb[0:4]; attn(1,*) needs h1 of
            # n=0 and vsb[4:8]; attn(2,*) needs q/k n=1 (keys to 1536) and
            # vsb[8:12]; attn(3,*) needs the rest of k n=1 and vsb[12:16].
            # out-proj units for query block group j unlock after attn(j,1).
            def F(fn, *a):
                return lambda: fn(*a)

            sched = {
                (0, 0): [F(proj_v_2sub, 0, 0),
                         F(proj_qk_half, wq_sb, qt, "cos", "sin", 1, 0, 0),
                         F(proj_qk_half, wk_sb, kt, "cos", "sin", 1, 0, 0),
                         F(proj_v_2sub, 0, 1)],
                (0, 1): [F(proj_qk_half, wq_sb, qt, "cos", "sin", 0, 0, 1),
                         F(proj_qk_half, wk_sb, kt, "cos", "sin", 0, 0, 1),
                         F(proj_qk_half, wq_sb, qt, "cos", "sin", 1, 0, 1),
                         F(proj_qk_half, wk_sb, kt, "cos", "sin", 1, 0, 1)],
                (1, 0): [F(proj_v_2sub, 0, 2), F(proj_v_2sub, 0, 3),
                         F(proj_qk_half, wq_sb, qt, "cos", "sin", 0, 1, 0),
                         F(proj_qk_half, wq_sb, qt, "cos", "sin", 1, 1, 0)],
                (1, 1): [F(proj_qk_half, wk_sb, kt, "cos", "sin", 0, 1, 0),
                         F(proj_qk_half, wk_sb, kt, "cos", "sin", 1, 1, 0),
                         F(proj_qk_half, wq_sb, qt, "cos", "sin", 0, 1, 1),
                         F(proj_qk_half, wq_sb, qt, "cos", "sin", 1, 1, 1)],
                (2, 0): [F(proj_v_2sub, 1, 0), F(proj_v_2sub, 1, 1),
                         F(proj_qk_half, wk_sb, kt, "cos", "sin", 0, 1, 1),
                         F(proj_qk_half, wk_sb, kt, "cos", "sin", 1, 1, 1),
                         F(emit_po_md, 0, 0, False), F(emit_po_md, 0, 1, False)],
                (2, 1): [F(proj_v_2sub, 1, 2), F(proj_v_2sub, 1, 3),
                         F(emit_po_md, 1, 0, False), F(emit_po_md, 1, 1, False),
                         F(emit_po_md, 2, 0, False), F(emit_po_md, 2, 1, False)],
                (3, 0): [F(emit_po_md, 3, 0, False), F(emit_po_md, 3, 1, False)]
                        + [F(emit_po_md, m, d, False)
                           for m in range(4, 7) for d in range(2)],
                (3, 1): [F(emit_po_md, m, d, False)
                         for m in range(7, 12) for d in range(2)],
            }

            for j in range(4):
                for p in range(2):
                    fillers.extend(sched[(j, p)])
                    nblocks = 4 * j + 4
                    attn_pair(j, p, len(fillers) / nblocks)
                    pump_all()

            # final out-proj for query blocks 12..15
            for m in range(12, 16):
                for dd in range(2):
                    emit_po_md(m, dd, True)

    nc.compile()
    return nc


def _host_inputs(x, Wq, Wk, Wv, Wo, token_positions):
    """Build per-core input maps (all host-side numpy prep)."""
    bf = ml_dtypes.bfloat16
    x = np.asarray(x, dtype=np.float32)
    Wq = np.asarray(Wq, dtype=np.float32)
    Wk = np.asarray(Wk, dtype=np.float32)
    Wv = np.asarray(Wv, dtype=np.float32)
    Wo = np.asarray(Wo, dtype=np.float32)
    pos = np.asarray(token_positions).astype(np.float64)

    # RoPE tables in the permuted-lane layout (16-lane e/o blocks).
    idx = np.arange(0, HD, 2, dtype=np.float64) / HD
    freqs = 1.0 / THETA ** idx                      # [32]
    ang = pos[:, None] * freqs[None, :]             # [S, 32]
    c, s = np.cos(ang).T, np.sin(ang).T             # [32, S]
    c64 = np.concatenate([c[0:16], c[0:16], c[16:32], c[16:32]], 0)
    s64 = np.concatenate([-s[0:16], s[0:16], -s[16:32], s[16:32]], 0)
    cosb = np.concatenate([c64, c64], 0).astype(bf)
    sinb = np.concatenate([s64, s64], 0).astype(bf)

    # 0/1 keep-mask for the diagonal block: keep keys (rows) <= query (cols),
    # duplicated for both heads of a pair.
    tri01 = (np.arange(128)[:, None] <= np.arange(128)[None, :]).astype(bf)
    trim = np.concatenate([tri01, tri01], axis=1)   # [128, 256]

    # per-head row permutation: [e0..e15, o0..o15, e16..e31, o16..o31]
    perm64 = np.concatenate([
        np.arange(0, 32, 2), np.arange(1, 32, 2),
        np.arange(32, 64, 2), np.arange(33, 64, 2),
    ])

    xts = [np.ascontiguousarray(x[b].T).astype(bf) for b in range(B)]

    in_maps = []
    for core in range(NCORES):
        b = core // 4
        heads = [4 * (core % 4) + hh for hh in range(HPC)]
        qk_rows = np.concatenate([g * HD + perm64 for g in heads])
        v_rows = np.concatenate([np.arange(g * HD, (g + 1) * HD) for g in heads])
        in_maps.append({
            "xt": xts[b],
            "wq": (np.ascontiguousarray(Wq[qk_rows, :].T) / np.sqrt(HD)).astype(bf),
            "wk": np.ascontiguousarray(Wk[qk_rows, :].T).astype(bf),
            "wv": np.ascontiguousarray(Wv[v_rows, :].T).astype(bf),
            "wo": np.ascontiguousarray(Wo[:, v_rows].T).astype(bf),
            "cos": cosb, "sin": sinb,
            "trim": trim,
        })
    return in_maps


def _ensure_ntff_hook():
    """Register the axon NTFF profile hook if the image's antenv lacks it."""
    import sys, types
    try:
        import antenv.axon_hooks  # noqa: F401
        return
    except ImportError:
        pass
    try:
        from trn_agent_boot.trn_boot import _ntff_profile_via_ctypes
        hook = _ntff_profile_via_ctypes("/opt/axon/libaxon_pjrt.so")
    except Exception:
        return
    mod = types.ModuleType("antenv.axon_hooks")
    mod.get_axon_ntff_profile_hook = lambda: hook
    mod.set_axon_ntff_profile_hook = lambda h: None
    sys.modules["antenv.axon_hooks"] = mod


def run(inputs, trace=False):
    """Run the SPMD kernel; returns (full_output, BassKernelResults)."""
    if trace:
        _ensure_ntff_hook()
    if "nc" not in _cached:
        _cached["nc"] = build_nc()
    nc = _cached["nc"]
    in_maps = _host_inputs(
        inputs["x"], inputs["Wq"], inputs["Wk"], inputs["Wv"], inputs["Wo"],
        inputs["token_positions"],
    )
    res = run_bass_kernel_spmd(nc, in_maps, core_ids=list(range(NCORES)),
                               trace=trace)
    out = np.zeros((B, S, D), dtype=np.float32)
    for core in range(NCORES):
        out[core // 4] += res.results[core]["out"].astype(np.float32)
    return out, res


def kernel(**inputs) -> np.ndarray:
    out, _ = run(inputs, trace=False)
    return out
